# revision 18
# baseline (speedup 1.0000x reference)
"""Bahdanau pointer-attention kernel v2 for Trainium2 (8 NeuronCores, SPMD).

energy[b, 1, n] = V . tanh(x[b, :N] @ W1.T + x[b, -1] @ W2.T)
B=32, N=2048, D=1024; data-parallel over batch across 8 cores.

v2 changes vs v1:
  - W1/W2 in bf16, x in fp8 E3M4 (fp8 rhs streams at bf16 speed on the PE;
    bf16 matmuls are ~2x faster than f32r on TRN2 HW). Input HBM traffic
    drops 40MB -> 12MB per core. rel err 1.28e-2 vs the 2e-2 gate.
  - chunk groups of G=2: one x DMA (2 MB) and one tanh ACT per (group, ec)
    spanning 2 PSUM banks -> half the DMA/ACT instruction count
  - packed V-dot (4 concurrent PE column-groups, ec-outer so adjacent
    matmuls share the stationary V column) + deferred emission so the PE
    never waits on ACT
  - three parallel DMA queues at startup: W2 slices on nc.scalar (gates
    only the query preamble), W1 slices on nc.gpsimd, x + query inputs on
    nc.sync -- the first keys matmul waits only on W1 + x chunk 0
  - optional tc.For_i hardware loop for reps (steady-state HW timing with
    constant NEFF size)

Measured (For_i rig, 2026-08-08): ~294-296us per pass steady state,
rel err 1.284e-2 (gate 2e-2). The keys matmul stream alone is ~267us with
DMA disabled (~261ns/MM: the sustained-load PE rate plus ~19ns/MM of
weight-load-path time), ACT/DMA/DVE fully hidden beneath it. Closed dead
ends, all measured: fp8 DoubleRow fails the accuracy gate (2.77e-2); fp8
weights lose precision on HW (1.79e-2) with no full-kernel speed gain;
walrus --enable-ldw-opt=true fails to compile; f32r is ~2x slower than
bf16; every buffer/grouping/loop-order knob A/B'd worse or neutral.
"""

from contextlib import ExitStack

import numpy as np
import ml_dtypes

import concourse.bass as bass
import concourse.mybir as mybir
import concourse.tile as tile
from concourse import bacc
from concourse.bass_utils import run_bass_kernel_spmd

B, N, D = 32, 2048, 1024
CORES = 8
BPC = B // CORES            # batches per core
NTOT = BPC * N              # 8192 key positions per core
P = 128
DC = D // P                 # 8 d-chunks (contraction)
EC = D // P                 # 8 e-chunks (output feature)
NT = 512                    # n tile (one PSUM bank of f32)
NCH = NTOT // NT            # 16 n-chunks per core
NPB = N // NT               # n-chunks per batch

f32 = mybir.dt.float32
f32r = mybir.dt.float32r
bf16 = mybir.dt.bfloat16

TRACE = False
LAST_EXEC_NS = None
LAST_RESULTS = None

_NC_CACHE = {}


def _body(ctx, tc, xT, xqT, w1T, w2T, vT, out, reps=1, hw_loop=False,
          do_dma=True, do_mm=True, do_act=True, do_vdot=True, G=2,
          x_bufs=3, kp_bufs=2, t_bufs=17, defer=1, j_outer=False,
          vdot_ec_outer=True, pack=True, w_dt=bf16, x_dt=bf16,
          w1_fp8=False):
    nc = tc.nc
    Tanh = mybir.ActivationFunctionType.Tanh
    NG = NCH // G  # groups per pass

    w_pool = ctx.enter_context(tc.tile_pool(name="w", bufs=1))
    x_pool = ctx.enter_context(tc.tile_pool(name="x", bufs=x_bufs))
    t_pool = ctx.enter_context(tc.tile_pool(name="tanh", bufs=t_bufs))
    small = ctx.enter_context(tc.tile_pool(name="small", bufs=1))
    en_pool = ctx.enter_context(tc.tile_pool(name="en", bufs=3))
    kpsum = ctx.enter_context(tc.tile_pool(name="kpsum", bufs=kp_bufs, space="PSUM"))
    vpsum = ctx.enter_context(tc.tile_pool(name="vpsum", bufs=2, space="PSUM"))

    # --- resident inputs on three parallel DMA queues: query inputs + x
    # on sync, W2 on scalar (per-ec-pair slices so the preamble starts
    # after ~512KB), W1 on gpsimd.
    xq_sb = small.tile([P, DC, BPC], x_dt, tag="xq")
    nc.sync.dma_start(xq_sb[:], xqT[:, :, :])
    v_sb = small.tile([P, EC], bf16, tag="v")
    nc.sync.dma_start(v_sb[:], vT[:, :])
    w2_sb = w_pool.tile([P, EC, DC, P], w_dt, tag="w2")
    for e0 in range(0, EC, 2):
        nc.scalar.dma_start(w2_sb[:, e0:e0 + 2, :, :],
                            w2T[:, e0:e0 + 2, :, :])
    # W1 rides the gpsimd (SWDGE) queue — a third DMA path independent of
    # the scalar queue (W2) and sync queue (x), so the first keys matmuls
    # don't wait behind W2 bytes.
    w1_dt = mybir.dt.float8e3 if w1_fp8 else w_dt
    w1_sb = w_pool.tile([P, DC, D], w1_dt, tag="w1")
    for d0 in range(0, DC, 2):
        nc.gpsimd.dma_start(w1_sb[:, d0:d0 + 2, :], w1T[:, d0:d0 + 2, :])
    # fp8 W1 is host-scaled by 32 into e3m4's normal range; the ACT undoes
    # it with its fused input scale: tanh(psum/32 + q).
    k_scale = (1.0 / 32.0) if w1_fp8 else 1.0

    # --- query preamble: q_sb[e128, (ec, b)] = x_query @ W2.T (transposed).
    # PSUM borrowed from the kpsum pool (preamble finishes before the main
    # loop needs sustained double-buffering).
    q_sb = small.tile([P, EC * BPC], f32, tag="q")
    for ec in range(EC):
        pq = kpsum.tile([P, G, NT], f32, tag="kp")
        # Col-tiled: 4 concurrent 32-column strips per (ec, dc) -> 32-col
        # weight loads (27ns vs 107ns) and 4-way PE concurrency, same
        # packing pattern as the V-dot.
        for dc in range(DC):
            for t in range(4):
                nc.tensor.matmul(
                    pq[32 * t:32 * (t + 1), 0, :BPC],
                    lhsT=w2_sb[:, ec, dc, 32 * t:32 * (t + 1)],
                    rhs=xq_sb[:, dc, :],
                    start=(dc == 0),
                    stop=(dc == DC - 1),
                    tile_position=(0, 32 * t),
                    skip_group_check=True,
                )
        nc.vector.tensor_copy(q_sb[:, ec * BPC:(ec + 1) * BPC], pq[:, 0, :BPC])

    x_fixed = None
    if not do_dma:
        x_fixed = x_pool.tile([P, DC, G * NT], x_dt, tag="x")
        nc.sync.dma_start(x_fixed[:], xT[0, :, :, :])

    def emit_vdot(ttl, g):
        pvs = {}
        if pack and vdot_ec_outer:
            # ec-outer: adjacent matmuls share the same stationary V column
            # (mirrors the keys-loop adjacency win); both chunks' pv banks
            # are filled in one sweep.
            for j in range(G):
                pvs[j] = vpsum.tile([P, NT], f32, tag="pv", name=f"pv{j}")
            for ec in range(EC):
                q4 = ec % 4
                for j in range(G):
                    nc.tensor.matmul(
                        pvs[j][32 * q4:32 * q4 + 1, :],
                        lhsT=v_sb[:, ec:ec + 1],
                        rhs=ttl[ec][:, j, :],
                        start=(ec < 4),
                        stop=(ec >= 4),
                        tile_position=(0, 32 * q4),
                        skip_group_check=True,
                    )
        for j in range(G):
            ch = g * G + j
            if pack:
                if vdot_ec_outer:
                    pv = pvs[j]
                else:
                    pv = vpsum.tile([P, NT], f32, tag="pv")
                    for ec in range(EC):
                        q4 = ec % 4
                        nc.tensor.matmul(
                            pv[32 * q4:32 * q4 + 1, :],
                            lhsT=v_sb[:, ec:ec + 1],
                            rhs=ttl[ec][:, j, :],
                            start=(ec < 4),
                            stop=(ec >= 4),
                            tile_position=(0, 32 * q4),
                            skip_group_check=True,
                        )
                # DVE may read at most one PSUM operand per TensorTensor:
                # copy one partial row out, then chain adds (1 PSUM input each).
                s0 = en_pool.tile([1, NT], f32, tag="s0")
                s1 = en_pool.tile([1, NT], f32, tag="s1")
                s2 = en_pool.tile([1, NT], f32, tag="s2")
                en = en_pool.tile([1, NT], f32, tag="en")
                nc.vector.tensor_copy(s0[:], pv[0:1, :])
                nc.vector.tensor_add(s1[:], s0[:], pv[32:33, :])
                nc.vector.tensor_add(s2[:], s1[:], pv[64:65, :])
                nc.vector.tensor_add(en[:], s2[:], pv[96:97, :])
            else:
                pv = vpsum.tile([P, NT], f32, tag="pv")
                for ec in range(EC):
                    nc.tensor.matmul(
                        pv[0:1, :],
                        lhsT=v_sb[:, ec:ec + 1],
                        rhs=ttl[ec][:, j, :],
                        start=(ec == 0),
                        stop=(ec == EC - 1),
                    )
                en = en_pool.tile([1, NT], f32, tag="en")
                nc.vector.tensor_copy(en[:], pv[0:1, :])
            nc.sync.dma_start(out[:, ch * NT:(ch + 1) * NT], en[:])

    def pass_body():
        pending = []
        for g in range(NG):
            if do_dma:
                x_sb = x_pool.tile([P, DC, G * NT], x_dt, tag="x")
                nc.sync.dma_start(x_sb[:], xT[g, :, :, :])
            else:
                x_sb = x_fixed
            if not do_mm:
                continue
            b = (g * G) // NPB
            ttl = []
            for ec in range(EC):
                kp = kpsum.tile([P, G, NT], f32, tag="kp")
                loop = ([(dc, j) for j in range(G) for dc in range(DC)]
                        if j_outer else
                        [(dc, j) for dc in range(DC) for j in range(G)])
                for dc, j in loop:
                    nc.tensor.matmul(
                        kp[:, j, :],
                        lhsT=w1_sb[:, dc, ec * P:(ec + 1) * P],
                        rhs=x_sb[:, dc, j * NT:(j + 1) * NT],
                        start=(dc == 0),
                        stop=(dc == DC - 1),
                    )
                # Emit the previous group's deferred V-dot right after this
                # group's first ec-block: its tanh inputs are complete by
                # then (no PE stall), and it keeps the V-dot off the tail of
                # the pass.
                if ec == 0 and do_act and do_vdot and len(pending) > defer - 1:
                    emit_vdot(*pending.pop(0))
                if not do_act:
                    continue
                tt = t_pool.tile([P, G, NT], bf16, tag="tt")
                nc.scalar.activation(
                    tt[:], kp[:], Tanh,
                    bias=q_sb[:, ec * BPC + b: ec * BPC + b + 1],
                    scale=k_scale,
                )
                ttl.append(tt)
            if not (do_act and do_vdot):
                continue
            pending.append((ttl, g))
        while pending:
            emit_vdot(*pending.pop(0))

    if hw_loop and reps > 1:
        with tc.For_i(0, reps):
            pass_body()
    else:
        for _ in range(reps):
            pass_body()


def build_module(reps=1, **opts):
    key = (reps, tuple(sorted(opts.items())))
    if key in _NC_CACHE:
        return _NC_CACHE[key]
    nc = bacc.Bacc("TRN2", target_bir_lowering=False, debug=False)
    w_dt = opts.get("w_dt", bf16)
    x_dt = opts.get("x_dt", bf16)
    G = opts.get("G", 2)
    # All inputs are host-packed to per-partition-contiguous layouts so each
    # DMA lowers to 128 large contiguous descriptors instead of thousands of
    # small ones: xT[g][p][dc][n], w1T[p][dc][e], w2T[p][ec][dc][e2].
    xT = nc.declare_dram_parameter("xT", [NCH // G, P, DC, G * NT], x_dt,
                                   isOutput=False)
    xqT = nc.declare_dram_parameter("xqT", [P, DC, BPC], x_dt, isOutput=False)
    w1_dt = mybir.dt.float8e3 if opts.get("w1_fp8") else w_dt
    w1T = nc.declare_dram_parameter("w1T", [P, DC, D], w1_dt, isOutput=False)
    w2T = nc.declare_dram_parameter("w2T", [P, EC, DC, P], w_dt, isOutput=False)
    vT = nc.declare_dram_parameter("vT", [P, EC], bf16, isOutput=False)
    out = nc.declare_dram_parameter("out", [1, NTOT], f32, isOutput=True)
    with tile.TileContext(nc) as tc:
        with ExitStack() as ctx:
            _body(ctx, tc, xT, xqT, w1T, w2T, vT, out, reps=reps, **opts)
    nc.compile()
    _NC_CACHE[key] = nc
    return nc


def shard_inputs(x, W1, W2, V, w_dt=np.dtype(ml_dtypes.bfloat16),
                 x_dt=np.dtype(ml_dtypes.bfloat16), G=2, w1_fp8=False):
    """Host-side sharding + per-partition-contiguous packing.

    Layouts match the kernel's SBUF tiles so every DMA is 128 large
    contiguous descriptors: xT[g][p][dc][n], w1T[p][dc][e],
    w2T[p][ec][dc][e2], xqT[p][dc][b].
    """
    x = np.asarray(x, dtype=np.float32)
    bf = ml_dtypes.bfloat16
    GNT = G * NT
    NG = NCH // G
    w1m = np.asarray(W1, np.float32).T.reshape(DC, P, D)
    if w1_fp8:
        w1m = w1m * 32.0
    w1_np = np.dtype(ml_dtypes.float8_e3m4) if w1_fp8 else w_dt
    w1T = np.ascontiguousarray(w1m.transpose(1, 0, 2)).astype(w1_np)  # [p,dc,e]
    w2m = np.asarray(W2, np.float32).T.reshape(DC, P, EC, P)
    w2T = np.ascontiguousarray(w2m.transpose(1, 2, 0, 3)).astype(w_dt)  # [p,ec,dc,e2]
    vT = np.ascontiguousarray(np.asarray(V, np.float32).reshape(EC, P).T).astype(bf)
    in_maps = []
    for c in range(CORES):
        xs = x[c * BPC:(c + 1) * BPC, :N, :]          # [BPC, N, D]
        xm = xs.transpose(2, 0, 1).reshape(DC, P, NG, GNT)
        xT = np.ascontiguousarray(xm.transpose(2, 1, 0, 3)).astype(x_dt)
        xq = x[c * BPC:(c + 1) * BPC, N, :]           # [BPC, D]
        xqT = np.ascontiguousarray(xq.T.reshape(DC, P, BPC).transpose(1, 0, 2)
                                   ).astype(x_dt)     # [p,dc,b]
        in_maps.append({
            "xT": xT, "xqT": xqT,
            "w1T": w1T, "w2T": w2T, "vT": vT,
        })
    return in_maps


FP8 = mybir.dt.float8e3


def kernel(x, W1, W2, V, city_count):
    global LAST_EXEC_NS, LAST_RESULTS
    assert int(city_count) == N
    nc = build_module(x_dt=FP8)
    in_maps = shard_inputs(x, W1, W2, V,
                           x_dt=np.dtype(ml_dtypes.float8_e3m4))
    res = run_bass_kernel_spmd(nc, in_maps, core_ids=list(range(CORES)),
                               trace=TRACE)
    LAST_EXEC_NS = res.exec_time_ns
    LAST_RESULTS = res
    out = np.concatenate(
        [res.results[c]["out"].reshape(BPC, N) for c in range(CORES)], axis=0
    )
    return out[:, None, :].astype(np.float32)
